# revision 16
# baseline (speedup 1.0000x reference)
"""TRN2 Bass kernel for nn_LocalAttention (B=4, T=2048, C=1024, window=16).

Sharding: 8 cores = (batch b, row-half h). Each core computes attention +
projections for its own 1024 rows (two 512-row chunks; h=0 gets global
chunks {0,3}, h=1 gets {1,2}; slot 0 = denser chunk).

Algebraic refactor (softmax is the only nonlinearity):
  S  = Q K^T = X (Wq^T Wk) X^T = X M X^T          M  host-precomputed
  Z  = (E V) Wo^T = (E X) (Wv^T Wo^T) = (E X) W2  W2 host-precomputed
so the K and V projections disappear entirely; the score sweep's
stationary operand is X^T itself and the value sweep's stationary
operand is X in natural layout - both direct DRAM inputs.

Precision: R/Z projection paths in fp32r; attention operands (R^T, X^T
key blocks, E^T, X value blocks) in bf16 (host-validated 3.1e-3 fro vs
2e-2 tolerance). bf16 halves the key/value DMA streams and E's SBUF.
PE rate is identical (1 cycle/row for both fp32r and bf16 at >=256
moving cols).

Per-core program (orientation: everything transposed, rows moving):
  R^T = M^T @ Xq^T                 [C, 1024]  (SBUF resident, bf16)
  S^T = (X^T_blk)^T @ R^T_chunk -> [keys, rows]; softmax-over-keys is a
        partition reduction done by a ones-vector matmul, and E^T feeds
  Y^T = X_blk^T @ E^T              [C, rows]  (X natural blocks)
  Z^T = (W2)^T @ Y^T               [C, rows]

Sparsity: mask keeps j >= i - 16 (reverse-causal), so each 512-row chunk's
kept key-block set is a SUFFIX {b..15}; processing key blocks in descending
order (position p -> block 15-p) makes every kept set a static PREFIX.
Chunk slot 0 runs 16 positions, slot 1 runs 9 - uniform across cores.
Only positions in MASKED_PS can have a non-trivial mask on ANY core
(boundary 16-key block or fully over-included block); all other
positions take exp() straight to bf16 E with no mask work at all.
Masked positions route exp through an f32 staging tile and multiply by
the data-driven is_ge mask (computed in f32: row indices up to 2047 are
not bf16-representable). Normalization is applied AFTER the out-proj
(linearity), as a single elementwise multiply on Z.

Stall engineering (from trace analysis):
- DMA queues start ~9.5us in (fixed runtime latency) and share ~330GB/s;
  only the critical path (m[0], xq chunk 0) runs ungated so the first
  R matmul starts ASAP. The X^T key-block stream (scalar queue) is
  gated behind xq0's arrival, and the W2 stream (gpsimd queue) behind
  the R phase, via tiny SBUF->DRAM dummy DMAs - otherwise they steal
  2/3 of startup bandwidth for data needed 40-90us later.
- M is loaded per-co-column into SEPARATE tiles (tile-granularity
  semaphores would otherwise make the first matmul wait for all of M).
- ktb/vco/wb pools are hoisted OUTSIDE the R-phase pools so their SBUF
  space never overlaps the R-phase M/xq space: the key/value streams
  need no false wait on the R phase drain.
"""
import numpy as np
import ml_dtypes

import concourse.bass as bass
import concourse.mybir as mybir
import concourse.tile as tile
from concourse import bacc
from concourse import bass_utils

N_CORES = 8
B, T, C = 4, 2048, 1024
WINDOW = 16
TOWN = T // 2          # own rows per core
CHUNK = 512            # rows per processing chunk
NCHUNK = TOWN // CHUNK  # 2
CI = C // 128          # 8 contraction blocks
CO = C // 128          # 8 output blocks
KB = T // 128          # 16 key blocks
SLOT_KBS = (16, 9)     # key-block positions per chunk slot (descending order)
CHUNK_MAP = {0: (0, 3), 1: (1, 2)}  # slot 0 = denser chunk
# Sweep positions where any core's mask is non-trivial. Block kb is
# trivially all-kept for chunk rows [r0, r0+511] iff 128*kb >= r0+495
# (the j >= i-16 boundary sweeps 4 blocks across each chunk's rows).
# Union over the two cores (h=0/h=1 chunk pairings):
#   slot0: h0 r0=0 -> kb<=3 (p>=12); h1 r0=512 -> kb<=7 (p>=8)
#   slot1: h0 r0=1536 -> all kb; h1 r0=1024 -> kb<=11 (p>=4)
MASKED_PS = {0: tuple(range(8, 16)), 1: tuple(range(0, 9))}
F32 = mybir.dt.float32
F32R = mybir.dt.float32r
BF16 = mybir.dt.bfloat16

_NC_CACHE = {}


def build():
    if "nc" in _NC_CACHE:
        return _NC_CACHE["nc"]
    nc = bacc.Bacc("TRN2", target_bir_lowering=False, debug=False,
                   num_devices=N_CORES)
    xt16 = nc.dram_tensor("xt16", [C, T], BF16, kind="ExternalInput").ap()
    xtq = nc.dram_tensor("xtq", [C, TOWN], F32R, kind="ExternalInput").ap()
    xn16 = nc.dram_tensor("xn16", [T, C], BF16, kind="ExternalInput").ap()
    m = nc.dram_tensor("m", [C, C], F32R, kind="ExternalInput").ap()
    w2 = nc.dram_tensor("w2", [C, C], F32R, kind="ExternalInput").ap()
    keyidx16 = nc.dram_tensor("keyidx16", [128, KB], F32, kind="ExternalInput").ap()
    rowidxb = nc.dram_tensor("rowidxb", [128, TOWN], F32, kind="ExternalInput").ap()
    zt = nc.dram_tensor("zt", [C, TOWN], F32, kind="ExternalOutput").ap()

    xt3 = xt16.rearrange("(ko ki) t -> ki ko t", ki=128)
    xtq3 = xtq.rearrange("(ko ki) t -> ki ko t", ki=128)
    xn3 = xn16.rearrange("(kb ki) c -> ki kb c", ki=128)
    m3 = m.rearrange("(ko ki) c -> ki ko c", ki=128)
    w23 = w2.rearrange("(ko ki) c -> ki ko c", ki=128)

    inv_sqrt_c = float(1.0 / np.sqrt(C))

    with tile.TileContext(nc) as tc:
        with tc.tile_pool(name="res", bufs=1) as res, \
             tc.tile_pool(name="ktb", bufs=8) as ktb_p, \
             tc.tile_pool(name="vco", bufs=8) as vsp, \
             tc.tile_pool(name="wb", bufs=2) as wb, \
             tc.tile_pool(name="zst", bufs=3) as zstp:
            rt_sb = res.tile([128, CI, TOWN], BF16, tag="rt")  # R^T resident
            w2_sb = res.tile([128, CI, C], F32R, tag="w2")
            ki16_sb = res.tile([128, KB], F32, tag="ki16")
            nc.gpsimd.dma_start(ki16_sb[:], keyidx16[:])
            ri_full = res.tile([128, TOWN], F32, tag="ri")
            nc.gpsimd.dma_start(ri_full[:], rowidxb[:])
            ones_row_f32 = res.tile([1, 128], F32, tag="onesrf")
            nc.vector.memset(ones_row_f32[:], 1.0)
            ones_1x128 = res.tile([1, 128], F32R, tag="o1")
            nc.vector.tensor_copy(ones_1x128[:], ones_row_f32[:])
            ones_col_f32 = res.tile([128, 1], F32, tag="onescf")
            nc.vector.memset(ones_col_f32[:], 1.0)
            ones_128x1 = res.tile([128, 1], BF16, tag="o2")
            nc.vector.tensor_copy(ones_128x1[:], ones_col_f32[:])

            # ===== R^T = M^T @ Xq^T (both chunks upfront) ===============
            with tc.tile_pool(name="mw", bufs=1) as mw, \
                 tc.tile_pool(name="xa", bufs=2) as xa, \
                 tc.tile_pool(name="ps_r", bufs=2, space="PSUM") as ps_r:
                m_t = [mw.tile([128, CI, 128], F32R, tag=f"m{co}",
                               name=f"m{co}")
                       for co in range(CO)]
                # Queue split by measured per-queue bandwidth (sync HW-DGE
                # ~275 GB/s, scalar ~131 GB/s): critical path m[0]+xq0 on
                # sync; the rest of M and xq1 interleaved across both in
                # consumption order.
                nc.sync.dma_start(m_t[0][:], m3[:, :, 0:128])
                xq0 = xa.tile([128, CI, CHUNK], F32R, tag="xa")
                nc.sync.dma_start(xq0[:], xtq3[:, :, 0:CHUNK])
                xq1 = xa.tile([128, CI, CHUNK], F32R, tag="xa")
                for co in (2, 4, 6):
                    nc.scalar.dma_start(m_t[co][:],
                                        m3[:, :, co * 128:(co + 1) * 128])
                nc.scalar.dma_start(xq1[:], xtq3[:, :, CHUNK:2 * CHUNK])
                for co in (1, 3, 5, 7):
                    nc.sync.dma_start(m_t[co][:],
                                      m3[:, :, co * 128:(co + 1) * 128])
                for qch in range(NCHUNK):
                    xq_sb = xq0 if qch == 0 else xq1
                    for co in range(CO):
                        rps = ps_r.tile([128, CHUNK], F32, tag="rps")
                        for ci in range(CI):
                            nc.tensor.matmul(
                                rps[:], m_t[co][:, ci, :],
                                xq_sb[:, ci, :], start=(ci == 0), stop=(ci == CI - 1))
                        with nc.allow_low_precision(reason="bf16 attention path"):
                            nc.vector.tensor_copy(
                                rt_sb[:, co, qch * CHUNK:(qch + 1) * CHUNK], rps[:])

            # w2 split across both HW queues AFTER the critical m/xq
            # descriptors (issue order = queue order); lands well before
            # the Z stage needs it
            for ci in range(CI):
                eng = nc.sync if ci % 2 == 0 else nc.scalar
                eng.dma_start(w2_sb[:, ci, :], w23[:, ci, :])

            # ===== attention + out-proj =================================
            with tc.tile_pool(name="et", bufs=1) as etp, \
                 tc.tile_pool(name="ysb", bufs=2) as ysb_p, \
                 tc.tile_pool(name="ps_s", bufs=3, space="PSUM") as ps_s, \
                 tc.tile_pool(name="ps_sh", bufs=1, space="PSUM") as ps_sh, \
                 tc.tile_pool(name="ps_y", bufs=2, space="PSUM") as ps_y, \
                 tc.tile_pool(name="ps_z", bufs=2, space="PSUM") as ps_z:
                for ch in range(NCHUNK):
                    nkb = SLOT_KBS[ch]
                    rsl = slice(ch * CHUNK, (ch + 1) * CHUNK)

                    et = etp.tile([128, KB, CHUNK], BF16, tag="et")
                    # --- sweep 1a: scores + exp (+ mask) (descending kb) ---
                    for p in range(nkb):
                        kb = KB - 1 - p
                        kt_b = ktb_p.tile([128, CI, 128], BF16, tag="ktb")
                        nc.sync.dma_start(
                            kt_b[:], xt3[:, :, kb * 128:(kb + 1) * 128])
                        sps = ps_s.tile([128, CHUNK], F32, tag="sps")
                        for ci in range(CI):
                            nc.tensor.matmul(
                                sps[:], kt_b[:, ci, :], rt_sb[:, ci, rsl],
                                start=(ci == 0), stop=(ci == CI - 1))
                        with nc.allow_low_precision(reason="bf16 attention path"):
                            if p in MASKED_PS[ch]:
                                etf = wb.tile([128, CHUNK], F32, tag="etf")
                                nc.scalar.activation(
                                    etf[:], sps[:],
                                    mybir.ActivationFunctionType.Exp,
                                    scale=inv_sqrt_c)
                                mask = wb.tile([128, CHUNK], F32, tag="mask")
                                nc.vector.tensor_tensor(
                                    mask[:],
                                    ki16_sb[:, kb:kb + 1].to_broadcast((128, CHUNK)),
                                    ri_full[:, rsl], mybir.AluOpType.is_ge)
                                nc.vector.tensor_tensor(
                                    et[:, p, :], etf[:], mask[:],
                                    mybir.AluOpType.mult)
                            else:
                                nc.scalar.activation(
                                    et[:, p, :], sps[:],
                                    mybir.ActivationFunctionType.Exp,
                                    scale=inv_sqrt_c)
                    # --- sweep 1b: key-sums via ones matmul ---
                    sums_ps = ps_sh.tile([1, CHUNK], F32, tag="shared")
                    for p in range(nkb):
                        nc.tensor.matmul(sums_ps[:], ones_128x1[:], et[:, p, :],
                                         start=(p == 0), stop=(p == nkb - 1))
                    recip = wb.tile([1, CHUNK], F32R, tag="recip")
                    with nc.allow_low_precision(reason="fp32r normalizer broadcast"):
                        nc.vector.reciprocal(recip[:], sums_ps[:])

                    # --- sweep 2: Y^T = X^T_blk @ E^T per cout block ---
                    y_sb = ysb_p.tile([128, CO, CHUNK], F32R, tag="ysb")
                    for co in range(CO):
                        v_co = vsp.tile([128, KB, 128], BF16, tag="vco")
                        nc.scalar.dma_start(
                            v_co[:, :nkb, :],
                            xn3[:, KB - nkb:, co * 128:(co + 1) * 128])
                        yps = ps_y.tile([128, CHUNK], F32, tag="yps")
                        for p in range(nkb):
                            nc.tensor.matmul(yps[:], v_co[:, nkb - 1 - p, :],
                                             et[:, p, :],
                                             start=(p == 0), stop=(p == nkb - 1))
                        nc.vector.tensor_copy(y_sb[:, co, :], yps[:])

                    # normalizer broadcast to 128 partitions (PE, cheap),
                    # placed after Y so the DVE reciprocal has the whole
                    # Y sweep to complete -> no PE stall
                    rb_ps = ps_sh.tile([128, CHUNK], F32, tag="shared")
                    nc.tensor.matmul(rb_ps[:], ones_1x128[:], recip[:],
                                     start=True, stop=True)
                    rb_sb = wb.tile([128, CHUNK], F32, tag="rbsb")
                    nc.vector.tensor_copy(rb_sb[:], rb_ps[:])

                    # --- out-proj + normalize ---
                    for co in range(CO):
                        zps = ps_z.tile([128, CHUNK], F32, tag="zps")
                        for ci in range(CI):
                            nc.tensor.matmul(
                                zps[:], w2_sb[:, ci, co * 128:(co + 1) * 128],
                                y_sb[:, ci, :], start=(ci == 0), stop=(ci == CI - 1))
                        zst = zstp.tile([128, CHUNK], F32, tag="zst")
                        nc.vector.tensor_tensor(zst[:], zps[:], rb_sb[:],
                                                mybir.AluOpType.mult)
                        nc.gpsimd.dma_start(zt[co * 128:(co + 1) * 128, rsl],
                                            zst[:])
    nc.compile()
    _NC_CACHE["nc"] = nc
    return nc


def make_in_maps(inputs):
    x = np.asarray(inputs["x"], dtype=np.float32)
    for bname in ("bq", "bk", "bv", "bo"):
        bval = np.asarray(inputs[bname])
        assert np.all(bval == 0.0), f"{bname} nonzero: unsupported fast path"
    wq = np.asarray(inputs["Wq"], np.float64)
    wk = np.asarray(inputs["Wk"], np.float64)
    wv = np.asarray(inputs["Wv"], np.float64)
    wo = np.asarray(inputs["Wo"], np.float64)
    m = np.ascontiguousarray((wq.T @ wk).astype(np.float32))
    w2 = np.ascontiguousarray((wv.T @ wo.T).astype(np.float32))
    keyidx16 = (np.arange(T, dtype=np.float32).reshape(KB, 128).T + WINDOW
                ).copy()  # [128, KB]
    in_maps = []
    for core in range(N_CORES):
        b, h = divmod(core, 2)
        xt_b = np.ascontiguousarray(x[b].T)  # [C, T]
        ch0, ch1 = CHUNK_MAP[h]
        xtq = np.concatenate(
            [xt_b[:, ch0 * CHUNK:(ch0 + 1) * CHUNK],
             xt_b[:, ch1 * CHUNK:(ch1 + 1) * CHUNK]], axis=1)
        rowidx = np.concatenate(
            [np.arange(ch0 * CHUNK, (ch0 + 1) * CHUNK, dtype=np.float32),
             np.arange(ch1 * CHUNK, (ch1 + 1) * CHUNK, dtype=np.float32)])
        rowidxb = np.ascontiguousarray(
            np.broadcast_to(rowidx[None, :], (128, TOWN)))
        in_maps.append({
            "xt16": np.ascontiguousarray(xt_b.astype(ml_dtypes.bfloat16)),
            "xtq": np.ascontiguousarray(xtq),
            "xn16": np.ascontiguousarray(x[b].astype(ml_dtypes.bfloat16)),
            "m": m, "w2": w2,
            "keyidx16": keyidx16, "rowidxb": rowidxb,
        })
    return in_maps


def gather_output(results, dtype):
    out = np.empty((B, T, C), dtype=dtype)
    for core in range(N_CORES):
        b, h = divmod(core, 2)
        y = results[core]["zt"].T  # [TOWN rows, C]
        ch0, ch1 = CHUNK_MAP[h]
        out[b, ch0 * CHUNK:(ch0 + 1) * CHUNK] = y[:CHUNK]
        out[b, ch1 * CHUNK:(ch1 + 1) * CHUNK] = y[CHUNK:]
    return out


def kernel(**inputs):
    nc = build()
    in_maps = make_in_maps(inputs)
    res = bass_utils.run_bass_kernel_spmd(nc, in_maps,
                                          core_ids=list(range(N_CORES)))
    return gather_output(res.results, np.asarray(inputs["x"]).dtype)


# revision 18
# speedup vs baseline: 1.1440x; 1.1440x over previous
"""TRN2 Bass kernel for nn_LocalAttention (B=4, T=2048, C=1024, window=16).

Sharding: 8 cores = (batch b, row-half h). Each core computes attention +
projections for its own 1024 rows (two 512-row chunks; h=0 gets global
chunks {0,3}, h=1 gets {1,2}; slot 0 = denser chunk).

Algebraic refactor (softmax is the only nonlinearity):
  S  = Q K^T = X (Wq^T Wk) X^T = X M X^T          M  host-precomputed
  Z  = (E V) Wo^T = (E X) (Wv^T Wo^T) = (E X) W2  W2 host-precomputed
so the K and V projections disappear entirely; the score sweep's
stationary operand is X^T itself and the value sweep's stationary
operand is X in natural layout - both direct DRAM inputs.

Precision: R/Z projection paths in fp32r; attention operands (R^T, X^T
key blocks, E^T, X value blocks) in bf16 (host-validated 3.1e-3 fro vs
2e-2 tolerance). bf16 halves the key/value DMA streams and E's SBUF.
PE rate is identical (1 cycle/row for both fp32r and bf16 at >=256
moving cols).

Per-core program (orientation: everything transposed, rows moving):
  R^T = M^T @ Xq^T                 [C, 1024]  (SBUF resident, bf16)
  S^T = (X^T_blk)^T @ R^T_chunk -> [keys, rows]; softmax-over-keys is a
        partition reduction done by a ones-vector matmul, and E^T feeds
  Y^T = X_blk^T @ E^T              [C, rows]  (X natural blocks)
  Z^T = (W2)^T @ Y^T               [C, rows]

Sparsity: mask keeps j >= i - 16 (reverse-causal), so each 512-row chunk's
kept key-block set is a SUFFIX {b..15}; processing key blocks in descending
order (position p -> block 15-p) makes every kept set a static PREFIX.
Chunk slot 0 runs 16 positions, slot 1 runs 9 - uniform across cores.
Only positions in MASKED_PS can have a non-trivial mask on ANY core
(boundary 16-key block or fully over-included block); all other
positions take exp() straight to bf16 E with no mask work at all.
Masked positions route exp through an f32 staging tile and multiply by
the data-driven is_ge mask (computed in f32: row indices up to 2047 are
not bf16-representable). Normalization is applied AFTER the out-proj
(linearity), as a single elementwise multiply on Z.

Stall engineering (from trace analysis):
- DMA queues start ~9.5us in (fixed runtime latency) and share ~330GB/s;
  only the critical path (m[0], xq chunk 0) runs ungated so the first
  R matmul starts ASAP. The X^T key-block stream (scalar queue) is
  gated behind xq0's arrival, and the W2 stream (gpsimd queue) behind
  the R phase, via tiny SBUF->DRAM dummy DMAs - otherwise they steal
  2/3 of startup bandwidth for data needed 40-90us later.
- M is loaded per-co-column into SEPARATE tiles (tile-granularity
  semaphores would otherwise make the first matmul wait for all of M).
- ktb/vco/wb pools are hoisted OUTSIDE the R-phase pools so their SBUF
  space never overlaps the R-phase M/xq space: the key/value streams
  need no false wait on the R phase drain.
"""
import numpy as np
import ml_dtypes

import concourse.bass as bass
import concourse.mybir as mybir
import concourse.tile as tile
from concourse import bacc
from concourse import bass_utils

N_CORES = 8
B, T, C = 4, 2048, 1024
WINDOW = 16
TOWN = T // 2          # own rows per core
CHUNK = 512            # rows per processing chunk
NCHUNK = TOWN // CHUNK  # 2
CI = C // 128          # 8 contraction blocks
CO = C // 128          # 8 output blocks
KB = T // 128          # 16 key blocks
SLOT_KBS = (16, 9)     # key-block positions per chunk slot (descending order)
CHUNK_MAP = {0: (0, 3), 1: (1, 2)}  # slot 0 = denser chunk
# Sweep positions where any core's mask is non-trivial. Block kb is
# trivially all-kept for chunk rows [r0, r0+511] iff 128*kb >= r0+495
# (the j >= i-16 boundary sweeps 4 blocks across each chunk's rows).
# Union over the two cores (h=0/h=1 chunk pairings):
#   slot0: h0 r0=0 -> kb<=3 (p>=12); h1 r0=512 -> kb<=7 (p>=8)
#   slot1: h0 r0=1536 -> all kb; h1 r0=1024 -> kb<=11 (p>=4)
MASKED_PS = {0: tuple(range(8, 16)), 1: tuple(range(0, 9))}
F32 = mybir.dt.float32
F32R = mybir.dt.float32r
BF16 = mybir.dt.bfloat16

_NC_CACHE = {}


def build():
    if "nc" in _NC_CACHE:
        return _NC_CACHE["nc"]
    nc = bacc.Bacc("TRN2", target_bir_lowering=False, debug=False,
                   num_devices=N_CORES)
    xt16 = nc.dram_tensor("xt16", [C, T], BF16, kind="ExternalInput").ap()
    xtq = nc.dram_tensor("xtq", [C, TOWN], F32R, kind="ExternalInput").ap()
    xn16 = nc.dram_tensor("xn16", [T, C], BF16, kind="ExternalInput").ap()
    m = nc.dram_tensor("m", [C, C], F32R, kind="ExternalInput").ap()
    w2 = nc.dram_tensor("w2", [C, C], F32R, kind="ExternalInput").ap()
    keyidx16 = nc.dram_tensor("keyidx16", [128, KB], F32, kind="ExternalInput").ap()
    rowidxb = nc.dram_tensor("rowidxb", [128, TOWN], F32, kind="ExternalInput").ap()
    zt = nc.dram_tensor("zt", [C, TOWN], F32, kind="ExternalOutput").ap()

    xt3 = xt16.rearrange("(ko ki) t -> ki ko t", ki=128)
    xtq3 = xtq.rearrange("(ko ki) t -> ki ko t", ki=128)
    xn3 = xn16.rearrange("(kb ki) c -> ki kb c", ki=128)
    m3 = m.rearrange("(ko ki) c -> ki ko c", ki=128)
    w23 = w2.rearrange("(ko ki) c -> ki ko c", ki=128)

    inv_sqrt_c = float(1.0 / np.sqrt(C))

    with tile.TileContext(nc) as tc:
        with tc.tile_pool(name="res", bufs=1) as res, \
             tc.tile_pool(name="ktb", bufs=8) as ktb_p, \
             tc.tile_pool(name="vco", bufs=8) as vsp, \
             tc.tile_pool(name="wb", bufs=2) as wb, \
             tc.tile_pool(name="zst", bufs=3) as zstp:
            rt_sb = res.tile([128, CI, TOWN], BF16, tag="rt")  # R^T resident
            w2_sb = res.tile([128, CI, C], F32R, tag="w2")
            ki16_sb = res.tile([128, KB], F32, tag="ki16")
            nc.gpsimd.dma_start(ki16_sb[:], keyidx16[:])
            ri_full = res.tile([128, TOWN], F32, tag="ri")
            nc.gpsimd.dma_start(ri_full[:], rowidxb[:])
            ones_row_f32 = res.tile([1, 128], F32, tag="onesrf")
            nc.vector.memset(ones_row_f32[:], 1.0)
            ones_1x128 = res.tile([1, 128], F32R, tag="o1")
            nc.vector.tensor_copy(ones_1x128[:], ones_row_f32[:])
            ones_col_f32 = res.tile([128, 1], F32, tag="onescf")
            nc.vector.memset(ones_col_f32[:], 1.0)
            ones_128x1 = res.tile([128, 1], BF16, tag="o2")
            nc.vector.tensor_copy(ones_128x1[:], ones_col_f32[:])

            # ===== R^T = M^T @ Xq^T (both chunks upfront) ===============
            with tc.tile_pool(name="mw", bufs=1) as mw, \
                 tc.tile_pool(name="xa", bufs=2) as xa, \
                 tc.tile_pool(name="ps_r", bufs=2, space="PSUM") as ps_r:
                m_t = [mw.tile([128, CI, 128], F32R, tag=f"m{co}",
                               name=f"m{co}")
                       for co in range(CO)]
                # Queue split by measured per-queue bandwidth (sync HW-DGE
                # ~275 GB/s, scalar ~131 GB/s): critical path m[0]+xq0 on
                # sync; the rest of M and xq1 interleaved across both in
                # consumption order.
                nc.sync.dma_start(m_t[0][:], m3[:, :, 0:128])
                # per-ci xq loads: the first R chain's matmul ci consumes
                # slice ci as it lands (slice-level dep tracking), so the
                # PE starts after m[0]+xq0[ci=0] (~0.75 MB) instead of
                # waiting for the full 2.5 MB front
                xq0 = xa.tile([128, CI, CHUNK], F32R, tag="xa")
                for ci in range(CI):
                    nc.sync.dma_start(xq0[:, ci, :], xtq3[:, ci, 0:CHUNK])
                xq1 = xa.tile([128, CI, CHUNK], F32R, tag="xa")
                for co in (2, 4, 6):
                    nc.scalar.dma_start(m_t[co][:],
                                        m3[:, :, co * 128:(co + 1) * 128])
                for ci in range(CI):
                    nc.scalar.dma_start(xq1[:, ci, :],
                                        xtq3[:, ci, CHUNK:2 * CHUNK])
                for co in (1, 3, 5, 7):
                    nc.sync.dma_start(m_t[co][:],
                                      m3[:, :, co * 128:(co + 1) * 128])
                for qch in range(NCHUNK):
                    xq_sb = xq0 if qch == 0 else xq1
                    for co in range(CO):
                        rps = ps_r.tile([128, CHUNK], F32, tag="rps")
                        for ci in range(CI):
                            nc.tensor.matmul(
                                rps[:], m_t[co][:, ci, :],
                                xq_sb[:, ci, :], start=(ci == 0), stop=(ci == CI - 1))
                        with nc.allow_low_precision(reason="bf16 attention path"):
                            nc.vector.tensor_copy(
                                rt_sb[:, co, qch * CHUNK:(qch + 1) * CHUNK], rps[:])

            # w2 split across both HW queues AFTER the critical m/xq
            # descriptors (issue order = queue order); lands well before
            # the Z stage needs it
            for ci in range(CI):
                eng = nc.sync if ci % 2 == 0 else nc.scalar
                eng.dma_start(w2_sb[:, ci, :], w23[:, ci, :])

            # ===== attention + out-proj =================================
            with tc.tile_pool(name="et", bufs=1) as etp, \
                 tc.tile_pool(name="ysb", bufs=2) as ysb_p, \
                 tc.tile_pool(name="ps_s", bufs=3, space="PSUM") as ps_s, \
                 tc.tile_pool(name="ps_sh", bufs=1, space="PSUM") as ps_sh, \
                 tc.tile_pool(name="ps_y", bufs=2, space="PSUM") as ps_y, \
                 tc.tile_pool(name="ps_z", bufs=2, space="PSUM") as ps_z:
                for ch in range(NCHUNK):
                    nkb = SLOT_KBS[ch]
                    rsl = slice(ch * CHUNK, (ch + 1) * CHUNK)

                    et = etp.tile([128, KB, CHUNK], BF16, tag="et")
                    # --- sweep 1a: scores + exp (+ mask) (descending kb) ---
                    for p in range(nkb):
                        kb = KB - 1 - p
                        kt_b = ktb_p.tile([128, CI, 128], BF16, tag="ktb")
                        nc.sync.dma_start(
                            kt_b[:], xt3[:, :, kb * 128:(kb + 1) * 128])
                        sps = ps_s.tile([128, CHUNK], F32, tag="sps")
                        for ci in range(CI):
                            nc.tensor.matmul(
                                sps[:], kt_b[:, ci, :], rt_sb[:, ci, rsl],
                                start=(ci == 0), stop=(ci == CI - 1))
                        with nc.allow_low_precision(reason="bf16 attention path"):
                            if p in MASKED_PS[ch]:
                                etf = wb.tile([128, CHUNK], F32, tag="etf")
                                nc.scalar.activation(
                                    etf[:], sps[:],
                                    mybir.ActivationFunctionType.Exp,
                                    scale=inv_sqrt_c)
                                mask = wb.tile([128, CHUNK], F32, tag="mask")
                                nc.vector.tensor_tensor(
                                    mask[:],
                                    ki16_sb[:, kb:kb + 1].to_broadcast((128, CHUNK)),
                                    ri_full[:, rsl], mybir.AluOpType.is_ge)
                                nc.vector.tensor_tensor(
                                    et[:, p, :], etf[:], mask[:],
                                    mybir.AluOpType.mult)
                            else:
                                nc.scalar.activation(
                                    et[:, p, :], sps[:],
                                    mybir.ActivationFunctionType.Exp,
                                    scale=inv_sqrt_c)
                    # --- sweep 1b: key-sums via ones matmul ---
                    sums_ps = ps_sh.tile([1, CHUNK], F32, tag="shared")
                    for p in range(nkb):
                        nc.tensor.matmul(sums_ps[:], ones_128x1[:], et[:, p, :],
                                         start=(p == 0), stop=(p == nkb - 1))
                    recip = wb.tile([1, CHUNK], F32R, tag="recip")
                    with nc.allow_low_precision(reason="fp32r normalizer broadcast"):
                        nc.vector.reciprocal(recip[:], sums_ps[:])

                    # --- sweep 2: Y^T = X^T_blk @ E^T per cout block ---
                    y_sb = ysb_p.tile([128, CO, CHUNK], F32R, tag="ysb")
                    for co in range(CO):
                        v_co = vsp.tile([128, KB, 128], BF16, tag="vco")
                        nc.scalar.dma_start(
                            v_co[:, :nkb, :],
                            xn3[:, KB - nkb:, co * 128:(co + 1) * 128])
                        yps = ps_y.tile([128, CHUNK], F32, tag="yps")
                        for p in range(nkb):
                            nc.tensor.matmul(yps[:], v_co[:, nkb - 1 - p, :],
                                             et[:, p, :],
                                             start=(p == 0), stop=(p == nkb - 1))
                        nc.vector.tensor_copy(y_sb[:, co, :], yps[:])

                    # normalizer broadcast to 128 partitions (PE, cheap),
                    # placed after Y so the DVE reciprocal has the whole
                    # Y sweep to complete -> no PE stall
                    rb_ps = ps_sh.tile([128, CHUNK], F32, tag="shared")
                    nc.tensor.matmul(rb_ps[:], ones_1x128[:], recip[:],
                                     start=True, stop=True)
                    rb_sb = wb.tile([128, CHUNK], F32, tag="rbsb")
                    nc.vector.tensor_copy(rb_sb[:], rb_ps[:])

                    # --- out-proj + normalize ---
                    for co in range(CO):
                        zps = ps_z.tile([128, CHUNK], F32, tag="zps")
                        for ci in range(CI):
                            nc.tensor.matmul(
                                zps[:], w2_sb[:, ci, co * 128:(co + 1) * 128],
                                y_sb[:, ci, :], start=(ci == 0), stop=(ci == CI - 1))
                        zst = zstp.tile([128, CHUNK], F32, tag="zst")
                        nc.vector.tensor_tensor(zst[:], zps[:], rb_sb[:],
                                                mybir.AluOpType.mult)
                        # ch0 outs ride scalar (slack after vco ch0; keeps
                        # sync free to prefetch ch1 key blocks); ch1 outs
                        # ride the fast sync queue so the final drain
                        # overlaps the Z pipeline
                        zeng = nc.scalar if ch == 0 else nc.sync
                        zeng.dma_start(zt[co * 128:(co + 1) * 128, rsl],
                                       zst[:])
    nc.compile()
    _NC_CACHE["nc"] = nc
    return nc


def make_in_maps(inputs):
    x = np.asarray(inputs["x"], dtype=np.float32)
    for bname in ("bq", "bk", "bv", "bo"):
        bval = np.asarray(inputs[bname])
        assert np.all(bval == 0.0), f"{bname} nonzero: unsupported fast path"
    wq = np.asarray(inputs["Wq"], np.float64)
    wk = np.asarray(inputs["Wk"], np.float64)
    wv = np.asarray(inputs["Wv"], np.float64)
    wo = np.asarray(inputs["Wo"], np.float64)
    m = np.ascontiguousarray((wq.T @ wk).astype(np.float32))
    w2 = np.ascontiguousarray((wv.T @ wo.T).astype(np.float32))
    keyidx16 = (np.arange(T, dtype=np.float32).reshape(KB, 128).T + WINDOW
                ).copy()  # [128, KB]
    in_maps = []
    for core in range(N_CORES):
        b, h = divmod(core, 2)
        xt_b = np.ascontiguousarray(x[b].T)  # [C, T]
        ch0, ch1 = CHUNK_MAP[h]
        xtq = np.concatenate(
            [xt_b[:, ch0 * CHUNK:(ch0 + 1) * CHUNK],
             xt_b[:, ch1 * CHUNK:(ch1 + 1) * CHUNK]], axis=1)
        rowidx = np.concatenate(
            [np.arange(ch0 * CHUNK, (ch0 + 1) * CHUNK, dtype=np.float32),
             np.arange(ch1 * CHUNK, (ch1 + 1) * CHUNK, dtype=np.float32)])
        rowidxb = np.ascontiguousarray(
            np.broadcast_to(rowidx[None, :], (128, TOWN)))
        in_maps.append({
            "xt16": np.ascontiguousarray(xt_b.astype(ml_dtypes.bfloat16)),
            "xtq": np.ascontiguousarray(xtq),
            "xn16": np.ascontiguousarray(x[b].astype(ml_dtypes.bfloat16)),
            "m": m, "w2": w2,
            "keyidx16": keyidx16, "rowidxb": rowidxb,
        })
    return in_maps


def gather_output(results, dtype):
    out = np.empty((B, T, C), dtype=dtype)
    for core in range(N_CORES):
        b, h = divmod(core, 2)
        y = results[core]["zt"].T  # [TOWN rows, C]
        ch0, ch1 = CHUNK_MAP[h]
        out[b, ch0 * CHUNK:(ch0 + 1) * CHUNK] = y[:CHUNK]
        out[b, ch1 * CHUNK:(ch1 + 1) * CHUNK] = y[CHUNK:]
    return out


def kernel(**inputs):
    nc = build()
    in_maps = make_in_maps(inputs)
    res = bass_utils.run_bass_kernel_spmd(nc, in_maps,
                                          core_ids=list(range(N_CORES)))
    return gather_output(res.results, np.asarray(inputs["x"]).dtype)


# revision 20
# speedup vs baseline: 1.1442x; 1.0002x over previous
"""TRN2 Bass kernel for nn_LocalAttention (B=4, T=2048, C=1024, window=16).

Sharding: 8 cores = (batch b, row-half h). Each core computes attention +
projections for its own 1024 rows (two 512-row chunks; h=0 gets global
chunks {0,3}, h=1 gets {1,2}; slot 0 = denser chunk).

Algebraic refactor (softmax is the only nonlinearity):
  S  = Q K^T = X (Wq^T Wk) X^T = X M X^T          M  host-precomputed
  Z  = (E V) Wo^T = (E X) (Wv^T Wo^T) = (E X) W2  W2 host-precomputed
so the K and V projections disappear entirely; the score sweep's
stationary operand is X^T itself and the value sweep's stationary
operand is X in natural layout - both direct DRAM inputs.

Precision: R/Z projection paths in fp32r; attention operands (R^T, X^T
key blocks, E^T, X value blocks) in bf16 (host-validated 3.1e-3 fro vs
2e-2 tolerance). bf16 halves the key/value DMA streams and E's SBUF.
PE rate is identical (1 cycle/row for both fp32r and bf16 at >=256
moving cols).

Per-core program (orientation: everything transposed, rows moving):
  R^T = M^T @ Xq^T                 [C, 1024]  (SBUF resident, bf16)
  S^T = (X^T_blk)^T @ R^T_chunk -> [keys, rows]; softmax-over-keys is a
        partition reduction done by a ones-vector matmul, and E^T feeds
  Y^T = X_blk^T @ E^T              [C, rows]  (X natural blocks)
  Z^T = (W2)^T @ Y^T               [C, rows]

Sparsity: mask keeps j >= i - 16 (reverse-causal), so each 512-row chunk's
kept key-block set is a SUFFIX {b..15}; processing key blocks in descending
order (position p -> block 15-p) makes every kept set a static PREFIX.
Chunk slot 0 runs 16 positions, slot 1 runs 9 - uniform across cores.
Only positions in MASKED_PS can have a non-trivial mask on ANY core
(boundary 16-key block or fully over-included block); all other
positions take exp() straight to bf16 E with no mask work at all.
Masked positions route exp through an f32 staging tile and multiply by
the data-driven is_ge mask (computed in f32: row indices up to 2047 are
not bf16-representable). Normalization is applied AFTER the out-proj
(linearity), as a single elementwise multiply on Z.

Stall engineering (from trace analysis):
- DMA queues start ~9.5us in (fixed runtime latency) and share ~330GB/s;
  only the critical path (m[0], xq chunk 0) runs ungated so the first
  R matmul starts ASAP. The X^T key-block stream (scalar queue) is
  gated behind xq0's arrival, and the W2 stream (gpsimd queue) behind
  the R phase, via tiny SBUF->DRAM dummy DMAs - otherwise they steal
  2/3 of startup bandwidth for data needed 40-90us later.
- M is loaded per-co-column into SEPARATE tiles (tile-granularity
  semaphores would otherwise make the first matmul wait for all of M).
- ktb/vco/wb pools are hoisted OUTSIDE the R-phase pools so their SBUF
  space never overlaps the R-phase M/xq space: the key/value streams
  need no false wait on the R phase drain.
"""
import numpy as np
import ml_dtypes

import concourse.bass as bass
import concourse.mybir as mybir
import concourse.tile as tile
from concourse import bacc
from concourse import bass_utils

N_CORES = 8
B, T, C = 4, 2048, 1024
WINDOW = 16
TOWN = T // 2          # own rows per core
CHUNK = 512            # rows per processing chunk
NCHUNK = TOWN // CHUNK  # 2
CI = C // 128          # 8 contraction blocks
CO = C // 128          # 8 output blocks
KB = T // 128          # 16 key blocks
SLOT_KBS = (16, 9)     # key-block positions per chunk slot (descending order)
CHUNK_MAP = {0: (0, 3), 1: (1, 2)}  # slot 0 = denser chunk
# Sweep positions where any core's mask is non-trivial. Block kb is
# trivially all-kept for chunk rows [r0, r0+511] iff 128*kb >= r0+495
# (the j >= i-16 boundary sweeps 4 blocks across each chunk's rows).
# Union over the two cores (h=0/h=1 chunk pairings):
#   slot0: h0 r0=0 -> kb<=3 (p>=12); h1 r0=512 -> kb<=7 (p>=8)
#   slot1: h0 r0=1536 -> all kb; h1 r0=1024 -> kb<=11 (p>=4)
MASKED_PS = {0: tuple(range(8, 16)), 1: tuple(range(0, 9))}
F32 = mybir.dt.float32
F32R = mybir.dt.float32r
BF16 = mybir.dt.bfloat16

_NC_CACHE = {}


def build():
    if "nc" in _NC_CACHE:
        return _NC_CACHE["nc"]
    nc = bacc.Bacc("TRN2", target_bir_lowering=False, debug=False,
                   num_devices=N_CORES)
    xt16 = nc.dram_tensor("xt16", [C, T], BF16, kind="ExternalInput").ap()
    xtq = nc.dram_tensor("xtq", [C, TOWN], F32R, kind="ExternalInput").ap()
    xn16 = nc.dram_tensor("xn16", [T, C], BF16, kind="ExternalInput").ap()
    m = nc.dram_tensor("m", [C, C], F32R, kind="ExternalInput").ap()
    w2 = nc.dram_tensor("w2", [C, C], F32R, kind="ExternalInput").ap()
    keyidx16 = nc.dram_tensor("keyidx16", [128, KB], F32, kind="ExternalInput").ap()
    rowidxb = nc.dram_tensor("rowidxb", [128, TOWN], F32, kind="ExternalInput").ap()
    zt = nc.dram_tensor("zt", [C, TOWN], F32, kind="ExternalOutput").ap()

    xt3 = xt16.rearrange("(ko ki) t -> ki ko t", ki=128)
    xtq3 = xtq.rearrange("(ko ki) t -> ki ko t", ki=128)
    xn3 = xn16.rearrange("(kb ki) c -> ki kb c", ki=128)
    m3 = m.rearrange("(ko ki) c -> ki ko c", ki=128)
    w23 = w2.rearrange("(ko ki) c -> ki ko c", ki=128)

    inv_sqrt_c = float(1.0 / np.sqrt(C))

    with tile.TileContext(nc) as tc:
        with tc.tile_pool(name="res", bufs=1) as res, \
             tc.tile_pool(name="ktb", bufs=8) as ktb_p, \
             tc.tile_pool(name="vco", bufs=8) as vsp, \
             tc.tile_pool(name="wb", bufs=2) as wb, \
             tc.tile_pool(name="zst", bufs=3) as zstp:
            rt_sb = res.tile([128, CI, TOWN], BF16, tag="rt")  # R^T resident
            w2_sb = res.tile([128, CI, C], F32R, tag="w2")
            ki16_sb = res.tile([128, KB], F32, tag="ki16")
            nc.gpsimd.dma_start(ki16_sb[:], keyidx16[:])
            ri_full = res.tile([128, TOWN], F32, tag="ri")
            nc.gpsimd.dma_start(ri_full[:], rowidxb[:])
            ones_row_f32 = res.tile([1, 128], F32, tag="onesrf")
            nc.vector.memset(ones_row_f32[:], 1.0)
            ones_1x128 = res.tile([1, 128], F32R, tag="o1")
            nc.vector.tensor_copy(ones_1x128[:], ones_row_f32[:])
            ones_col_f32 = res.tile([128, 1], F32, tag="onescf")
            nc.vector.memset(ones_col_f32[:], 1.0)
            ones_128x1 = res.tile([128, 1], BF16, tag="o2")
            nc.vector.tensor_copy(ones_128x1[:], ones_col_f32[:])

            # ===== R^T = M^T @ Xq^T (both chunks upfront) ===============
            with tc.tile_pool(name="mw", bufs=1) as mw, \
                 tc.tile_pool(name="xa", bufs=2) as xa, \
                 tc.tile_pool(name="ps_r", bufs=2, space="PSUM") as ps_r:
                m_t = [mw.tile([128, CI, 128], F32R, tag=f"m{co}",
                               name=f"m{co}")
                       for co in range(CO)]
                # Queue split by measured per-queue bandwidth (sync HW-DGE
                # ~275 GB/s, scalar ~131 GB/s): critical path m[0]+xq0 on
                # sync; the rest of M and xq1 interleaved across both in
                # consumption order.
                nc.sync.dma_start(m_t[0][:], m3[:, :, 0:128])
                # per-ci xq loads: the first R chain's matmul ci consumes
                # slice ci as it lands (slice-level dep tracking), so the
                # PE starts after m[0]+xq0[ci=0] (~0.75 MB) instead of
                # waiting for the full 2.5 MB front
                xq0 = xa.tile([128, CI, CHUNK], F32R, tag="xa")
                for ci in range(CI):
                    nc.sync.dma_start(xq0[:, ci, :], xtq3[:, ci, 0:CHUNK])
                xq1 = xa.tile([128, CI, CHUNK], F32R, tag="xa")
                for co in (2, 4, 6):
                    nc.scalar.dma_start(m_t[co][:],
                                        m3[:, :, co * 128:(co + 1) * 128])
                for ci in range(4, CI):
                    nc.scalar.dma_start(xq1[:, ci, :],
                                        xtq3[:, ci, CHUNK:2 * CHUNK])
                for co in (1, 3, 5, 7):
                    nc.sync.dma_start(m_t[co][:],
                                      m3[:, :, co * 128:(co + 1) * 128])
                for ci in range(4):
                    nc.sync.dma_start(xq1[:, ci, :],
                                      xtq3[:, ci, CHUNK:2 * CHUNK])
                for qch in range(NCHUNK):
                    xq_sb = xq0 if qch == 0 else xq1
                    for co in range(CO):
                        rps = ps_r.tile([128, CHUNK], F32, tag="rps")
                        for ci in range(CI):
                            nc.tensor.matmul(
                                rps[:], m_t[co][:, ci, :],
                                xq_sb[:, ci, :], start=(ci == 0), stop=(ci == CI - 1))
                        with nc.allow_low_precision(reason="bf16 attention path"):
                            nc.vector.tensor_copy(
                                rt_sb[:, co, qch * CHUNK:(qch + 1) * CHUNK], rps[:])

            # w2 split across both HW queues AFTER the critical m/xq
            # descriptors (issue order = queue order); lands well before
            # the Z stage needs it
            for ci in range(CI):
                eng = nc.sync if ci % 2 == 0 else nc.scalar
                eng.dma_start(w2_sb[:, ci, :], w23[:, ci, :])

            # ===== attention + out-proj =================================
            with tc.tile_pool(name="et", bufs=1) as etp, \
                 tc.tile_pool(name="ysb", bufs=2) as ysb_p, \
                 tc.tile_pool(name="ps_s", bufs=3, space="PSUM") as ps_s, \
                 tc.tile_pool(name="ps_sh", bufs=1, space="PSUM") as ps_sh, \
                 tc.tile_pool(name="ps_y", bufs=2, space="PSUM") as ps_y, \
                 tc.tile_pool(name="ps_z", bufs=2, space="PSUM") as ps_z:
                for ch in range(NCHUNK):
                    nkb = SLOT_KBS[ch]
                    rsl = slice(ch * CHUNK, (ch + 1) * CHUNK)

                    et = etp.tile([128, KB, CHUNK], BF16, tag="et")
                    # --- sweep 1a: scores + exp (+ mask) (descending kb) ---
                    for p in range(nkb):
                        kb = KB - 1 - p
                        kt_b = ktb_p.tile([128, CI, 128], BF16, tag="ktb")
                        # first two ch0 blocks ride the idle gpsimd queue so
                        # the S sweep starts the moment the R phase ends;
                        # the sync queue (busy with m/xq until ~R end)
                        # sustains the rest
                        kteng = nc.gpsimd if (ch == 0 and p < 2) else nc.sync
                        kteng.dma_start(
                            kt_b[:], xt3[:, :, kb * 128:(kb + 1) * 128])
                        sps = ps_s.tile([128, CHUNK], F32, tag="sps")
                        for ci in range(CI):
                            nc.tensor.matmul(
                                sps[:], kt_b[:, ci, :], rt_sb[:, ci, rsl],
                                start=(ci == 0), stop=(ci == CI - 1))
                        with nc.allow_low_precision(reason="bf16 attention path"):
                            if p in MASKED_PS[ch]:
                                etf = wb.tile([128, CHUNK], F32, tag="etf")
                                nc.scalar.activation(
                                    etf[:], sps[:],
                                    mybir.ActivationFunctionType.Exp,
                                    scale=inv_sqrt_c)
                                mask = wb.tile([128, CHUNK], F32, tag="mask")
                                nc.vector.tensor_tensor(
                                    mask[:],
                                    ki16_sb[:, kb:kb + 1].to_broadcast((128, CHUNK)),
                                    ri_full[:, rsl], mybir.AluOpType.is_ge)
                                nc.vector.tensor_tensor(
                                    et[:, p, :], etf[:], mask[:],
                                    mybir.AluOpType.mult)
                            else:
                                nc.scalar.activation(
                                    et[:, p, :], sps[:],
                                    mybir.ActivationFunctionType.Exp,
                                    scale=inv_sqrt_c)
                    # --- sweep 1b: key-sums via ones matmul ---
                    sums_ps = ps_sh.tile([1, CHUNK], F32, tag="shared")
                    for p in range(nkb):
                        nc.tensor.matmul(sums_ps[:], ones_128x1[:], et[:, p, :],
                                         start=(p == 0), stop=(p == nkb - 1))
                    recip = wb.tile([1, CHUNK], F32R, tag="recip")
                    with nc.allow_low_precision(reason="fp32r normalizer broadcast"):
                        nc.vector.reciprocal(recip[:], sums_ps[:])

                    # --- sweep 2: Y^T = X^T_blk @ E^T per cout block ---
                    y_sb = ysb_p.tile([128, CO, CHUNK], F32R, tag="ysb")
                    for co in range(CO):
                        v_co = vsp.tile([128, KB, 128], BF16, tag="vco")
                        nc.scalar.dma_start(
                            v_co[:, :nkb, :],
                            xn3[:, KB - nkb:, co * 128:(co + 1) * 128])
                        yps = ps_y.tile([128, CHUNK], F32, tag="yps")
                        for p in range(nkb):
                            nc.tensor.matmul(yps[:], v_co[:, nkb - 1 - p, :],
                                             et[:, p, :],
                                             start=(p == 0), stop=(p == nkb - 1))
                        nc.vector.tensor_copy(y_sb[:, co, :], yps[:])

                    # normalizer broadcast to 128 partitions (PE, cheap),
                    # placed after Y so the DVE reciprocal has the whole
                    # Y sweep to complete -> no PE stall
                    rb_ps = ps_sh.tile([128, CHUNK], F32, tag="shared")
                    nc.tensor.matmul(rb_ps[:], ones_1x128[:], recip[:],
                                     start=True, stop=True)
                    rb_sb = wb.tile([128, CHUNK], F32, tag="rbsb")
                    nc.vector.tensor_copy(rb_sb[:], rb_ps[:])

                    # --- out-proj + normalize ---
                    for co in range(CO):
                        zps = ps_z.tile([128, CHUNK], F32, tag="zps")
                        for ci in range(CI):
                            nc.tensor.matmul(
                                zps[:], w2_sb[:, ci, co * 128:(co + 1) * 128],
                                y_sb[:, ci, :], start=(ci == 0), stop=(ci == CI - 1))
                        zst = zstp.tile([128, CHUNK], F32, tag="zst")
                        nc.vector.tensor_tensor(zst[:], zps[:], rb_sb[:],
                                                mybir.AluOpType.mult)
                        # ch0 outs ride scalar (slack after vco ch0; keeps
                        # sync free to prefetch ch1 key blocks); ch1 outs
                        # ride the fast sync queue so the final drain
                        # overlaps the Z pipeline
                        zeng = nc.scalar if ch == 0 else nc.sync
                        zeng.dma_start(zt[co * 128:(co + 1) * 128, rsl],
                                       zst[:])
    nc.compile()
    _NC_CACHE["nc"] = nc
    return nc


def make_in_maps(inputs):
    x = np.asarray(inputs["x"], dtype=np.float32)
    for bname in ("bq", "bk", "bv", "bo"):
        bval = np.asarray(inputs[bname])
        assert np.all(bval == 0.0), f"{bname} nonzero: unsupported fast path"
    wq = np.asarray(inputs["Wq"], np.float64)
    wk = np.asarray(inputs["Wk"], np.float64)
    wv = np.asarray(inputs["Wv"], np.float64)
    wo = np.asarray(inputs["Wo"], np.float64)
    m = np.ascontiguousarray((wq.T @ wk).astype(np.float32))
    w2 = np.ascontiguousarray((wv.T @ wo.T).astype(np.float32))
    keyidx16 = (np.arange(T, dtype=np.float32).reshape(KB, 128).T + WINDOW
                ).copy()  # [128, KB]
    in_maps = []
    for core in range(N_CORES):
        b, h = divmod(core, 2)
        xt_b = np.ascontiguousarray(x[b].T)  # [C, T]
        ch0, ch1 = CHUNK_MAP[h]
        xtq = np.concatenate(
            [xt_b[:, ch0 * CHUNK:(ch0 + 1) * CHUNK],
             xt_b[:, ch1 * CHUNK:(ch1 + 1) * CHUNK]], axis=1)
        rowidx = np.concatenate(
            [np.arange(ch0 * CHUNK, (ch0 + 1) * CHUNK, dtype=np.float32),
             np.arange(ch1 * CHUNK, (ch1 + 1) * CHUNK, dtype=np.float32)])
        rowidxb = np.ascontiguousarray(
            np.broadcast_to(rowidx[None, :], (128, TOWN)))
        in_maps.append({
            "xt16": np.ascontiguousarray(xt_b.astype(ml_dtypes.bfloat16)),
            "xtq": np.ascontiguousarray(xtq),
            "xn16": np.ascontiguousarray(x[b].astype(ml_dtypes.bfloat16)),
            "m": m, "w2": w2,
            "keyidx16": keyidx16, "rowidxb": rowidxb,
        })
    return in_maps


def gather_output(results, dtype):
    out = np.empty((B, T, C), dtype=dtype)
    for core in range(N_CORES):
        b, h = divmod(core, 2)
        y = results[core]["zt"].T  # [TOWN rows, C]
        ch0, ch1 = CHUNK_MAP[h]
        out[b, ch0 * CHUNK:(ch0 + 1) * CHUNK] = y[:CHUNK]
        out[b, ch1 * CHUNK:(ch1 + 1) * CHUNK] = y[CHUNK:]
    return out


def kernel(**inputs):
    nc = build()
    in_maps = make_in_maps(inputs)
    res = bass_utils.run_bass_kernel_spmd(nc, in_maps,
                                          core_ids=list(range(N_CORES)))
    return gather_output(res.results, np.asarray(inputs["x"]).dtype)


# revision 21
# speedup vs baseline: 1.2791x; 1.1179x over previous
"""TRN2 Bass kernel for nn_LocalAttention (B=4, T=2048, C=1024, window=16).

Sharding: 8 cores = (batch b, row-half h). Each core computes attention +
projections for its own 1024 rows (two 512-row chunks; h=0 gets global
chunks {0,3}, h=1 gets {1,2}; slot 0 = denser chunk).

Algebraic refactor (softmax is the only nonlinearity):
  S  = Q K^T = X (Wq^T Wk) X^T = X M X^T          M  host-precomputed
  Z  = (E V) Wo^T = (E X) (Wv^T Wo^T) = (E X) W2  W2 host-precomputed
so the K and V projections disappear entirely; the score sweep's
stationary operand is X^T itself and the value sweep's stationary
operand is X in natural layout - both direct DRAM inputs.

Precision: R/Z projection paths in fp32r; attention operands (R^T, X^T
key blocks, E^T, X value blocks) in bf16 (host-validated 3.1e-3 fro vs
2e-2 tolerance). bf16 halves the key/value DMA streams and E's SBUF.
PE rate is identical (1 cycle/row for both fp32r and bf16 at >=256
moving cols).

Per-core program (orientation: everything transposed, rows moving):
  R^T = M^T @ Xq^T                 [C, 1024]  (SBUF resident, bf16)
  S^T = (X^T_blk)^T @ R^T_chunk -> [keys, rows]; softmax-over-keys is a
        partition reduction done by a ones-vector matmul, and E^T feeds
  Y^T = X_blk^T @ E^T              [C, rows]  (X natural blocks)
  Z^T = (W2)^T @ Y^T               [C, rows]

Sparsity: mask keeps j >= i - 16 (reverse-causal), so each 512-row chunk's
kept key-block set is a SUFFIX {b..15}; processing key blocks in descending
order (position p -> block 15-p) makes every kept set a static PREFIX.
Chunk slot 0 runs 16 positions, slot 1 runs 9 - uniform across cores.
Only positions in MASKED_PS can have a non-trivial mask on ANY core
(boundary 16-key block or fully over-included block); all other
positions take exp() straight to bf16 E with no mask work at all.
Masked positions route exp through an f32 staging tile and multiply by
the data-driven is_ge mask (computed in f32: row indices up to 2047 are
not bf16-representable). Normalization is applied AFTER the out-proj
(linearity), as a single elementwise multiply on Z.

Stall engineering (from trace analysis):
- DMA queues start ~9.5us in (fixed runtime latency) and share ~330GB/s;
  only the critical path (m[0], xq chunk 0) runs ungated so the first
  R matmul starts ASAP. The X^T key-block stream (scalar queue) is
  gated behind xq0's arrival, and the W2 stream (gpsimd queue) behind
  the R phase, via tiny SBUF->DRAM dummy DMAs - otherwise they steal
  2/3 of startup bandwidth for data needed 40-90us later.
- M is loaded per-co-column into SEPARATE tiles (tile-granularity
  semaphores would otherwise make the first matmul wait for all of M).
- ktb/vco/wb pools are hoisted OUTSIDE the R-phase pools so their SBUF
  space never overlaps the R-phase M/xq space: the key/value streams
  need no false wait on the R phase drain.
"""
import numpy as np
import ml_dtypes

import concourse.bass as bass
import concourse.mybir as mybir
import concourse.tile as tile
from concourse import bacc
from concourse import bass_utils

N_CORES = 8
B, T, C = 4, 2048, 1024
WINDOW = 16
TOWN = T // 2          # own rows per core
CHUNK = 512            # rows per processing chunk
NCHUNK = TOWN // CHUNK  # 2
CI = C // 128          # 8 contraction blocks
CO = C // 128          # 8 output blocks
KB = T // 128          # 16 key blocks
SLOT_KBS = (16, 9)     # key-block positions per chunk slot (descending order)
CHUNK_MAP = {0: (0, 3), 1: (1, 2)}  # slot 0 = denser chunk
# Sweep positions where any core's mask is non-trivial. Block kb is
# trivially all-kept for chunk rows [r0, r0+511] iff 128*kb >= r0+495
# (the j >= i-16 boundary sweeps 4 blocks across each chunk's rows).
# Union over the two cores (h=0/h=1 chunk pairings):
#   slot0: h0 r0=0 -> kb<=3 (p>=12); h1 r0=512 -> kb<=7 (p>=8)
#   slot1: h0 r0=1536 -> all kb; h1 r0=1024 -> kb<=11 (p>=4)
MASKED_PS = {0: tuple(range(8, 16)), 1: tuple(range(0, 9))}
F32 = mybir.dt.float32
F32R = mybir.dt.float32r
BF16 = mybir.dt.bfloat16

_NC_CACHE = {}


def build():
    if "nc" in _NC_CACHE:
        return _NC_CACHE["nc"]
    nc = bacc.Bacc("TRN2", target_bir_lowering=False, debug=False,
                   num_devices=N_CORES)
    xt16 = nc.dram_tensor("xt16", [C, T], BF16, kind="ExternalInput").ap()
    xtq = nc.dram_tensor("xtq16", [C, TOWN], BF16, kind="ExternalInput").ap()
    xn16 = nc.dram_tensor("xn16", [T, C], BF16, kind="ExternalInput").ap()
    m = nc.dram_tensor("m16", [C, C], BF16, kind="ExternalInput").ap()
    w2 = nc.dram_tensor("w216", [C, C], BF16, kind="ExternalInput").ap()
    keyidx16 = nc.dram_tensor("keyidx16", [128, KB], F32, kind="ExternalInput").ap()
    rowidxb = nc.dram_tensor("rowidxb", [128, TOWN], F32, kind="ExternalInput").ap()
    zt = nc.dram_tensor("zt", [C, TOWN], F32, kind="ExternalOutput").ap()

    xt3 = xt16.rearrange("(ko ki) t -> ki ko t", ki=128)
    xtq3 = xtq.rearrange("(ko ki) t -> ki ko t", ki=128)
    xn3 = xn16.rearrange("(kb ki) c -> ki kb c", ki=128)
    m3 = m.rearrange("(ko ki) c -> ki ko c", ki=128)
    w23 = w2.rearrange("(ko ki) c -> ki ko c", ki=128)

    inv_sqrt_c = float(1.0 / np.sqrt(C))

    with tile.TileContext(nc) as tc:
        with tc.tile_pool(name="res", bufs=1) as res, \
             tc.tile_pool(name="ktb", bufs=8) as ktb_p, \
             tc.tile_pool(name="vco", bufs=8) as vsp, \
             tc.tile_pool(name="wb", bufs=2) as wb, \
             tc.tile_pool(name="zst", bufs=3) as zstp:
            rt_sb = res.tile([128, CI, TOWN], BF16, tag="rt")  # R^T resident
            w2_sb = res.tile([128, CI, C], BF16, tag="w2")
            ki16_sb = res.tile([128, KB], F32, tag="ki16")
            nc.gpsimd.dma_start(ki16_sb[:], keyidx16[:])
            ri_full = res.tile([128, TOWN], F32, tag="ri")
            nc.gpsimd.dma_start(ri_full[:], rowidxb[:])
            ones_row_f32 = res.tile([1, 128], F32, tag="onesrf")
            nc.vector.memset(ones_row_f32[:], 1.0)
            ones_1x128 = res.tile([1, 128], F32R, tag="o1")
            nc.vector.tensor_copy(ones_1x128[:], ones_row_f32[:])
            ones_col_f32 = res.tile([128, 1], F32, tag="onescf")
            nc.vector.memset(ones_col_f32[:], 1.0)
            ones_128x1 = res.tile([128, 1], BF16, tag="o2")
            nc.vector.tensor_copy(ones_128x1[:], ones_col_f32[:])

            # ===== R^T = M^T @ Xq^T (both chunks upfront) ===============
            with tc.tile_pool(name="mw", bufs=1) as mw, \
                 tc.tile_pool(name="xa", bufs=2) as xa, \
                 tc.tile_pool(name="ps_r", bufs=2, space="PSUM") as ps_r:
                m_t = [mw.tile([128, CI, 128], BF16, tag=f"m{co}",
                               name=f"m{co}")
                       for co in range(CO)]
                # Queue split by measured per-queue bandwidth (sync HW-DGE
                # ~275 GB/s, scalar ~131 GB/s): critical path m[0]+xq0 on
                # sync; the rest of M and xq1 interleaved across both in
                # consumption order.
                nc.sync.dma_start(m_t[0][:], m3[:, :, 0:128])
                # per-ci xq loads: the first R chain's matmul ci consumes
                # slice ci as it lands (slice-level dep tracking), so the
                # PE starts after m[0]+xq0[ci=0] (~0.75 MB) instead of
                # waiting for the full 2.5 MB front
                xq0 = xa.tile([128, CI, CHUNK], BF16, tag="xa")
                for ci in range(CI):
                    nc.sync.dma_start(xq0[:, ci, :], xtq3[:, ci, 0:CHUNK])
                xq1 = xa.tile([128, CI, CHUNK], BF16, tag="xa")
                for co in (2, 4, 6):
                    nc.scalar.dma_start(m_t[co][:],
                                        m3[:, :, co * 128:(co + 1) * 128])
                for ci in range(4, CI):
                    nc.scalar.dma_start(xq1[:, ci, :],
                                        xtq3[:, ci, CHUNK:2 * CHUNK])
                for co in (1, 3, 5, 7):
                    nc.sync.dma_start(m_t[co][:],
                                      m3[:, :, co * 128:(co + 1) * 128])
                for ci in range(4):
                    nc.sync.dma_start(xq1[:, ci, :],
                                      xtq3[:, ci, CHUNK:2 * CHUNK])
                for qch in range(NCHUNK):
                    xq_sb = xq0 if qch == 0 else xq1
                    for co in range(CO):
                        rps = ps_r.tile([128, CHUNK], F32, tag="rps")
                        for ci in range(CI):
                            nc.tensor.matmul(
                                rps[:], m_t[co][:, ci, :],
                                xq_sb[:, ci, :], start=(ci == 0), stop=(ci == CI - 1))
                        with nc.allow_low_precision(reason="bf16 attention path"):
                            nc.vector.tensor_copy(
                                rt_sb[:, co, qch * CHUNK:(qch + 1) * CHUNK], rps[:])

            # w2 split across both HW queues AFTER the critical m/xq
            # descriptors (issue order = queue order); lands well before
            # the Z stage needs it
            for ci in range(CI):
                eng = nc.sync if ci % 2 == 0 else nc.scalar
                eng.dma_start(w2_sb[:, ci, :], w23[:, ci, :])

            # ===== attention + out-proj =================================
            with tc.tile_pool(name="et", bufs=1) as etp, \
                 tc.tile_pool(name="ysb", bufs=2) as ysb_p, \
                 tc.tile_pool(name="ps_s", bufs=3, space="PSUM") as ps_s, \
                 tc.tile_pool(name="ps_sh", bufs=1, space="PSUM") as ps_sh, \
                 tc.tile_pool(name="ps_y", bufs=2, space="PSUM") as ps_y, \
                 tc.tile_pool(name="ps_z", bufs=2, space="PSUM") as ps_z:
                for ch in range(NCHUNK):
                    nkb = SLOT_KBS[ch]
                    rsl = slice(ch * CHUNK, (ch + 1) * CHUNK)

                    et = etp.tile([128, KB, CHUNK], BF16, tag="et")
                    # --- sweep 1a: scores + exp (+ mask) (descending kb) ---
                    for p in range(nkb):
                        kb = KB - 1 - p
                        kt_b = ktb_p.tile([128, CI, 128], BF16, tag="ktb")
                        # first two ch0 blocks ride the idle gpsimd queue so
                        # the S sweep starts the moment the R phase ends;
                        # the sync queue (busy with m/xq until ~R end)
                        # sustains the rest
                        kteng = nc.gpsimd if (ch == 0 and p < 2) else nc.sync
                        kteng.dma_start(
                            kt_b[:], xt3[:, :, kb * 128:(kb + 1) * 128])
                        sps = ps_s.tile([128, CHUNK], F32, tag="sps")
                        for ci in range(CI):
                            nc.tensor.matmul(
                                sps[:], kt_b[:, ci, :], rt_sb[:, ci, rsl],
                                start=(ci == 0), stop=(ci == CI - 1))
                        with nc.allow_low_precision(reason="bf16 attention path"):
                            if p in MASKED_PS[ch]:
                                etf = wb.tile([128, CHUNK], F32, tag="etf")
                                nc.scalar.activation(
                                    etf[:], sps[:],
                                    mybir.ActivationFunctionType.Exp,
                                    scale=inv_sqrt_c)
                                mask = wb.tile([128, CHUNK], F32, tag="mask")
                                nc.vector.tensor_tensor(
                                    mask[:],
                                    ki16_sb[:, kb:kb + 1].to_broadcast((128, CHUNK)),
                                    ri_full[:, rsl], mybir.AluOpType.is_ge)
                                nc.vector.tensor_tensor(
                                    et[:, p, :], etf[:], mask[:],
                                    mybir.AluOpType.mult)
                            else:
                                nc.scalar.activation(
                                    et[:, p, :], sps[:],
                                    mybir.ActivationFunctionType.Exp,
                                    scale=inv_sqrt_c)
                    # --- sweep 1b: key-sums via ones matmul ---
                    sums_ps = ps_sh.tile([1, CHUNK], F32, tag="shared")
                    for p in range(nkb):
                        nc.tensor.matmul(sums_ps[:], ones_128x1[:], et[:, p, :],
                                         start=(p == 0), stop=(p == nkb - 1))
                    recip = wb.tile([1, CHUNK], F32R, tag="recip")
                    with nc.allow_low_precision(reason="fp32r normalizer broadcast"):
                        nc.vector.reciprocal(recip[:], sums_ps[:])

                    # --- sweep 2: Y^T = X^T_blk @ E^T per cout block ---
                    y_sb = ysb_p.tile([128, CO, CHUNK], BF16, tag="ysb")
                    for co in range(CO):
                        v_co = vsp.tile([128, KB, 128], BF16, tag="vco")
                        nc.scalar.dma_start(
                            v_co[:, :nkb, :],
                            xn3[:, KB - nkb:, co * 128:(co + 1) * 128])
                        yps = ps_y.tile([128, CHUNK], F32, tag="yps")
                        for p in range(nkb):
                            nc.tensor.matmul(yps[:], v_co[:, nkb - 1 - p, :],
                                             et[:, p, :],
                                             start=(p == 0), stop=(p == nkb - 1))
                        nc.vector.tensor_copy(y_sb[:, co, :], yps[:])

                    # normalizer broadcast to 128 partitions (PE, cheap),
                    # placed after Y so the DVE reciprocal has the whole
                    # Y sweep to complete -> no PE stall
                    rb_ps = ps_sh.tile([128, CHUNK], F32, tag="shared")
                    nc.tensor.matmul(rb_ps[:], ones_1x128[:], recip[:],
                                     start=True, stop=True)
                    rb_sb = wb.tile([128, CHUNK], F32, tag="rbsb")
                    nc.vector.tensor_copy(rb_sb[:], rb_ps[:])

                    # --- out-proj + normalize ---
                    for co in range(CO):
                        zps = ps_z.tile([128, CHUNK], F32, tag="zps")
                        for ci in range(CI):
                            nc.tensor.matmul(
                                zps[:], w2_sb[:, ci, co * 128:(co + 1) * 128],
                                y_sb[:, ci, :], start=(ci == 0), stop=(ci == CI - 1))
                        zst = zstp.tile([128, CHUNK], F32, tag="zst")
                        nc.vector.tensor_tensor(zst[:], zps[:], rb_sb[:],
                                                mybir.AluOpType.mult)
                        # ch0 outs ride scalar (slack after vco ch0; keeps
                        # sync free to prefetch ch1 key blocks); ch1 outs
                        # ride the fast sync queue so the final drain
                        # overlaps the Z pipeline
                        zeng = nc.scalar if ch == 0 else nc.sync
                        zeng.dma_start(zt[co * 128:(co + 1) * 128, rsl],
                                       zst[:])
    nc.compile()
    _NC_CACHE["nc"] = nc
    return nc


def make_in_maps(inputs):
    x = np.asarray(inputs["x"], dtype=np.float32)
    for bname in ("bq", "bk", "bv", "bo"):
        bval = np.asarray(inputs[bname])
        assert np.all(bval == 0.0), f"{bname} nonzero: unsupported fast path"
    wq = np.asarray(inputs["Wq"], np.float64)
    wk = np.asarray(inputs["Wk"], np.float64)
    wv = np.asarray(inputs["Wv"], np.float64)
    wo = np.asarray(inputs["Wo"], np.float64)
    m = np.ascontiguousarray((wq.T @ wk).astype(np.float32))
    w2 = np.ascontiguousarray((wv.T @ wo.T).astype(np.float32))
    keyidx16 = (np.arange(T, dtype=np.float32).reshape(KB, 128).T + WINDOW
                ).copy()  # [128, KB]
    in_maps = []
    for core in range(N_CORES):
        b, h = divmod(core, 2)
        xt_b = np.ascontiguousarray(x[b].T)  # [C, T]
        ch0, ch1 = CHUNK_MAP[h]
        xtq = np.concatenate(
            [xt_b[:, ch0 * CHUNK:(ch0 + 1) * CHUNK],
             xt_b[:, ch1 * CHUNK:(ch1 + 1) * CHUNK]], axis=1)
        rowidx = np.concatenate(
            [np.arange(ch0 * CHUNK, (ch0 + 1) * CHUNK, dtype=np.float32),
             np.arange(ch1 * CHUNK, (ch1 + 1) * CHUNK, dtype=np.float32)])
        rowidxb = np.ascontiguousarray(
            np.broadcast_to(rowidx[None, :], (128, TOWN)))
        in_maps.append({
            "xt16": np.ascontiguousarray(xt_b.astype(ml_dtypes.bfloat16)),
            "xtq16": np.ascontiguousarray(xtq.astype(ml_dtypes.bfloat16)),
            "xn16": np.ascontiguousarray(x[b].astype(ml_dtypes.bfloat16)),
            "m16": m.astype(ml_dtypes.bfloat16),
            "w216": w2.astype(ml_dtypes.bfloat16),
            "keyidx16": keyidx16, "rowidxb": rowidxb,
        })
    return in_maps


def gather_output(results, dtype):
    out = np.empty((B, T, C), dtype=dtype)
    for core in range(N_CORES):
        b, h = divmod(core, 2)
        y = results[core]["zt"].T  # [TOWN rows, C]
        ch0, ch1 = CHUNK_MAP[h]
        out[b, ch0 * CHUNK:(ch0 + 1) * CHUNK] = y[:CHUNK]
        out[b, ch1 * CHUNK:(ch1 + 1) * CHUNK] = y[CHUNK:]
    return out


def kernel(**inputs):
    nc = build()
    in_maps = make_in_maps(inputs)
    res = bass_utils.run_bass_kernel_spmd(nc, in_maps,
                                          core_ids=list(range(N_CORES)))
    return gather_output(res.results, np.asarray(inputs["x"]).dtype)
